# revision 28
# baseline (speedup 1.0000x reference)
"""Multi-head graph attention (GAT-style) Trainium2 Bass kernel, v5.

Full-input contract: kernel(**inputs) takes the complete arrays, shards
batch-wise across 8 NeuronCores (2 batches each), and gathers the output.

Math per batch b, head h (KD=16 head dim):
  Q = h @ Wq_h, K = h @ Wk_h, V = h @ Wv_h            [N, 16]
  compatT[m, n] = (K Q^T)[m, n]                        [N, N] (transposed)
  p = exp(0.25 * compatT) * adjT                       (mask after exp; exact:
      masked entries are exactly 0, matching softmax(-inf) * adj)
  headsT[v, n] = (V'.T @ p)  with V' = [V | 1 | 0*15]  -> row 16 = denominator
  out[n, :] = sum_h (headsT_h / denom_h).T @ Wout_h + h[n, :]

Schedule: the ScalarE exp stream (128 x [128,1024] activations per core,
~1.15us each) is the roofline; everything else hides behind it.
  - compat matmuls 4-way ROW-TILED (contraction 16): heads j=0..3 of group g
    run concurrently in 32-row PE groups (tile_position=(32j,0)); Q/K are
    projected DIRECTLY into gapped layout (head 4g+j at partitions 32j..+15)
    using host-side zero-padded bf16 weights — no shift DMAs.
  - PSUM: ring of 3 x [128,1024] compat tiles (6 banks) + 2 PV banks.
  - loop: b -> g (head group) -> nh (n half) -> mc; PV accumulates over mc
    into pv[g] (col-tiled, head j at partitions 32j, 32-wide with zero pad).
  - finish pipelined per n-half: denominator gather into a [128,32] all-lane
    layout -> reciprocal -> DRAM roundtrip broadcast -> bf16 normalize ->
    bf16 output matmul in the idle PV bank + residual add -> half store.
    Emission is phase-split (pending slots keyed to later loop iterations) so
    slow DMA chains never head-of-line-block the DVE/PE streams.
  - all input loads issued upfront; next batch's projections emitted in three
    chunks mid-current-batch so the exp stream never waits at transitions.

Precision: inputs to all matmuls are bf16 (PSUM accumulation fp32); the
residual add and softmax normalization stay fp32.
"""

import numpy as np
import ml_dtypes
from contextlib import ExitStack

import concourse.bass as bass
import concourse.mybir as mybir
import concourse.tile as tile
from concourse.bass_utils import run_bass_kernel_spmd

B, N, E, H, KD = 16, 1024, 128, 8, 16
CORES = 8
BPC = B // CORES  # batches per core
F32 = mybir.dt.float32
BF16 = mybir.dt.bfloat16
MC = N // 128  # number of 128-row chunks of m
NH = 2         # n halves
NHS = N // NH  # 512
VW = 32        # padded per-head V' width (16 vals + 1 ones + 15 zeros)


def build_kernel():
    nc = bass.Bass()
    hT_d = nc.dram_tensor("ht", [BPC, E, N], BF16, kind="ExternalInput")
    h_d = nc.dram_tensor("hn", [BPC, N, E], F32, kind="ExternalInput")
    adjt_d = nc.dram_tensor("adjt", [BPC, N, 2 * N], BF16, kind="ExternalInput")
    wall_d = nc.dram_tensor("wall", [128, 7, 128], BF16, kind="ExternalInput")
    out_d = nc.dram_tensor("out", [BPC, N, E], F32, kind="ExternalOutput")

    with ExitStack() as ctx:
        tc = ctx.enter_context(tile.TileContext(nc))
        consts = ctx.enter_context(tc.tile_pool(name="consts", bufs=1))
        io_pool = ctx.enter_context(tc.tile_pool(name="io", bufs=2))
        qk_pool = ctx.enter_context(tc.tile_pool(name="qk", bufs=2))
        pt_pool = ctx.enter_context(tc.tile_pool(name="pt", bufs=8))
        pm_pool = ctx.enter_context(tc.tile_pool(name="pm", bufs=8))
        hu_pool = ctx.enter_context(tc.tile_pool(name="hu", bufs=2))
        fin_pool = ctx.enter_context(tc.tile_pool(name="fin", bufs=2))
        ob_pool = ctx.enter_context(tc.tile_pool(name="ob", bufs=2))
        ps_pool = ctx.enter_context(tc.tile_pool(name="ps", bufs=3, space="PSUM"))
        pv_pool = ctx.enter_context(tc.tile_pool(name="pv", bufs=1, space="PSUM"))
        dram = ctx.enter_context(tc.tile_pool(name="dram", bufs=2, space="DRAM"))

        wall_sb = consts.tile([128, 7, 128], BF16, tag="wall")
        wq_sb = [wall_sb[:, g, :] for g in range(2)]
        wk_sb = [wall_sb[:, 2 + g, :] for g in range(2)]
        wo_sb = [wall_sb[:, 4 + g, :] for g in range(2)]
        wv_sb = wall_sb[:, 6, :]

        ones32 = consts.tile([128, 32], BF16, tag="ones32")
        nc.vector.memset(ones32, 1.0)

        # V' tiles with the constant cols preset once (ones col + zero pad);
        # per-batch projections overwrite only cols 0:KD.
        v_fix = []
        for i in range(2):
            vt = consts.tile([128, MC, H, VW], BF16, tag=f"vfix{i}")
            nc.vector.memset(vt[:, :, :, KD : KD + 1], 1.0)
            nc.vector.memset(vt[:, :, :, KD + 1 : VW], 0.0)
            v_fix.append(vt)

        # ---- all input loads upfront (SP HW queue); ordering favors the
        # batch-0 critical path: hT0 + first adj chunk + weights first.
        hT_sb, h_sb, adjT_sb = [], [], []
        for bb in range(BPC):
            ht = io_pool.tile([E, N], BF16, tag="ht", name="ht")
            adj = io_pool.tile([128, MC, 2 * N], BF16, tag="adj", name="adj")
            hn = io_pool.tile([128, MC, E], F32, tag="hn", name="hn")
            hT_sb.append(ht)
            adjT_sb.append(adj)
            h_sb.append(hn)
        nc.sync.dma_start(out=hT_sb[0], in_=hT_d[0, :, :])
        nc.sync.dma_start(out=wall_sb, in_=wall_d[:, :, :])
        nc.sync.dma_start(out=adjT_sb[0][:, 0, :], in_=adjt_d[0, 0:128, :])
        for mc_ in range(1, MC):
            nc.sync.dma_start(
                out=adjT_sb[0][:, mc_, :],
                in_=adjt_d[0, mc_ * 128 : (mc_ + 1) * 128, :],
            )
        nc.sync.dma_start(
            out=h_sb[0], in_=h_d[0].rearrange("(c p) e -> p c e", p=128)
        )
        for bb in range(1, BPC):
            nc.sync.dma_start(out=hT_sb[bb], in_=hT_d[bb, :, :])
            for mc_ in range(MC):
                nc.sync.dma_start(
                    out=adjT_sb[bb][:, mc_, :],
                    in_=adjt_d[bb, mc_ * 128 : (mc_ + 1) * 128, :],
                )
            nc.sync.dma_start(
                out=h_sb[bb], in_=h_d[bb].rearrange("(c p) e -> p c e", p=128)
            )

        qg_all = {}   # (b, g) -> gapped Q tile
        kg_all = {}
        v_all = {}    # b -> v_nat

        def _proj_one(b, g, w_list, store, tagc):
            def p1():
                p_ps = ps_pool.tile([128, N], F32, tag="ring", name="ring")
                for nt in range(2):
                    nc.tensor.matmul(
                        out=p_ps[:, nt * NHS : (nt + 1) * NHS],
                        lhsT=w_list[g],
                        rhs=hT_sb[b][:, nt * NHS : (nt + 1) * NHS],
                        start=True,
                        stop=True,
                    )
                gt = qk_pool.tile([128, N], BF16, tag=f"{tagc}g{g}", name=f"{tagc}g{g}")
                nc.vector.tensor_copy(out=gt, in_=p_ps)
                store[(b, g)] = gt

            return [p1]

        def project_qk(b, g):
            return _proj_one(b, g, wq_sb, qg_all, "q") + _proj_one(
                b, g, wk_sb, kg_all, "k"
            )

        def project_v(b):
            def p1():
                v_ps = ps_pool.tile([128, N], F32, tag="ring", name="ring")
                for mc_ in range(MC):
                    nc.tensor.matmul(
                        out=v_ps[:, mc_ * 128 : (mc_ + 1) * 128],
                        lhsT=hT_sb[b][:, mc_ * 128 : (mc_ + 1) * 128],
                        rhs=wv_sb,
                        start=True,
                        stop=True,
                    )
                v_nat = v_fix[b % 2]
                nc.vector.tensor_copy(
                    out=v_nat[:, :, :, 0:KD],
                    in_=v_ps.rearrange("p (c h k) -> p c h k", h=H, k=KD),
                )
                v_all[b] = v_nat

            return [p1]

        # finish state per (b, g): rbt broadcast tiles per half
        rbts = {}

        def finish_part1(b, g, half, hu, fast_tail=False):
            """denominator gather + reciprocal + broadcast for one n-half.
            dd layout: partition 64*half+16*j+pp, free i (32),
            n = 512*half + 32*pp + i. Normal path broadcasts via a DRAM
            roundtrip into rbt; the tail path (last batch g1) instead gathers
            rec rows to [4,N] and PE-outer-products them into the idle pv0
            bank (shorter latency; ACT queue helps with the DMAs)."""
            key = (b, g)
            if key not in rbts:
                rbts[key] = {
                    "dd": fin_pool.tile([128, 32], F32, tag=f"dd{g}", name=f"dd{g}"),
                }
                if not fast_tail:
                    rbts[key]["rd"] = dram.tile([4, N], F32, tag=f"recd{g}", name=f"recd{g}")
                    rbts[key]["rbt"] = fin_pool.tile([128, N], F32, tag=f"rb{g}", name=f"rb{g}")
            st = rbts[key]
            dd = st["dd"]
            rd, rbt = st.get("rd"), st.get("rbt")
            hs = slice(half * NHS, (half + 1) * NHS)
            post_stream = fast_tail and half == 1
            for j in range(4):
                eng = nc.scalar if (post_stream and j % 2) else nc.sync
                eng.dma_start(
                    out=dd[64 * half + 16 * j : 64 * half + 16 * j + 16, :],
                    in_=hu[g][32 * j + KD : 32 * j + KD + 1, hs].rearrange(
                        "o (p i) -> o p i", p=16
                    ),
                )
            rec = fin_pool.tile([128, 32], F32, tag=f"rec{g}", name=f"rec{g}", bufs=1)
            nc.vector.reciprocal(
                out=rec[64 * half : 64 * half + 64, :],
                in_=dd[64 * half : 64 * half + 64, :],
            )
            if fast_tail:
                recb = fin_pool.tile([128, 32], BF16, tag="recb", name="recb", bufs=1)
                nc.vector.tensor_copy(
                    out=recb[64 * half : 64 * half + 64, :],
                    in_=rec[64 * half : 64 * half + 64, :],
                )
                rec4 = fin_pool.tile([128, NHS], BF16, tag="rec4", name="rec4", bufs=1)
                for j in range(4):
                    eng = nc.scalar if (post_stream and j % 2) else nc.sync
                    eng.dma_start(
                        out=rec4[32 * j : 32 * j + 1, :].rearrange(
                            "o (p i) -> o p i", p=16
                        ),
                        in_=recb[64 * half + 16 * j : 64 * half + 16 * j + 16, :],
                    )
                rbc = pv_pool.tile([128, NHS], F32, tag="pv0", name="rbc")
                for j in range(4):
                    nc.tensor.matmul(
                        out=rbc[32 * j : 32 * j + 32, :],
                        lhsT=ones32[32 * j : 32 * j + 1, :],
                        rhs=rec4[32 * j : 32 * j + 1, :],
                        start=True,
                        stop=True,
                        tile_position=(32 * j, 32 * j),
                    )
                st[f"rbc{half}"] = rbc
                return
            nc.sync.dma_start(
                out=bass.AP(
                    tensor=rd.tensor,
                    offset=rd.offset + half * NHS,
                    ap=[[N, 4], [32, 16], [1, 32]],
                ),
                in_=rec[64 * half : 64 * half + 64, :],
            )
            for j in range(4):
                src = rd[j : j + 1, hs]
                bc = bass.AP(
                    tensor=src.tensor,
                    offset=src.offset,
                    ap=[[0, 32]] + list(src.ap[1:]),
                )
                nc.sync.dma_start(out=rbt[32 * j : 32 * (j + 1), hs], in_=bc)

        def finish_part2(b, g, half, hu, ob, ccs=None, tail=False):
            """normalize (bf16) + output matmul (in the idle pv bank(s)) +
            residual add (g==0) / combine (g==1) into the per-batch ob tile.
            ccs selects a subset of output chunks (normalize runs only when
            the first chunk of the half is included)."""
            hs = slice(half * NHS, (half + 1) * NHS)
            if ccs is None:
                ccs = list(range(4 * half, 4 * half + 4))
            hun = rbts[(b, g)].setdefault(
                "hun",
                fin_pool.tile([128, N], BF16, tag=f"hun{g}", name=f"hun{g}"),
            )
            if 4 * half in ccs:
                rbc = rbts[(b, g)].pop(f"rbc{half}", None)
                if rbc is not None:
                    nc.vector.tensor_mul(hun[:, hs], hu[g][:, hs], rbc)
                else:
                    rbt = rbts[(b, g)]["rbt"]
                    nc.vector.tensor_mul(hun[:, hs], hu[g][:, hs], rbt[:, hs])
            if tail:
                o_pvs = [
                    pv_pool.tile([128, NHS], F32, tag="pv0", name="opv0"),
                    pv_pool.tile([128, NHS], F32, tag="pv1", name="opv1"),
                ]
            else:
                o_pvs = [pv_pool.tile([128, NHS], F32, tag=f"pv{g}", name=f"pv{g}")]
            for i, cc in enumerate(ccs):
                o_pv = o_pvs[i % len(o_pvs)]
                reg = slice((cc % 4) * E, (cc % 4) * E + E)
                nc.tensor.matmul(
                    out=o_pv[:, reg],
                    lhsT=hun[:, cc * 128 : (cc + 1) * 128],
                    rhs=wo_sb[g],
                    start=True,
                    stop=True,
                )
                if g == 0:
                    nc.vector.tensor_add(
                        ob[:, cc, :], o_pv[:, reg], h_sb[b][:, cc, :]
                    )
                else:
                    nc.vector.tensor_add(ob[:, cc, :], o_pv[:, reg], ob[:, cc, :])
            if g == 1 and 4 * half + 3 in ccs:
                nc.sync.dma_start(
                    out=out_d[b, half * NHS : (half + 1) * NHS, :].rearrange(
                        "(c p) e -> p c e", p=128
                    ),
                    in_=ob[:, 4 * half : 4 * half + 4, :],
                )

        # deferred-emission queue: at most one small piece fires per mc
        # iteration, keeping injected PE/DVE work under the per-stage slack
        from collections import deque
        work_q = deque()

        def run_attention_group(b, g, hu, ob):
            qg, kg, v_nat = qg_all[(b, g)], kg_all[(b, g)], v_all[b]
            for nh in range(NH):
                ns = slice(nh * NHS, (nh + 1) * NHS)
                pvt = pv_pool.tile([128, NHS], F32, tag=f"pv{g}", name=f"pv{g}")

                def emit_pv(pms, mc_, pvt=pvt, g=g, v_nat=v_nat):
                    for j in range(4):
                        nc.tensor.matmul(
                            out=pvt[32 * j : 32 * j + VW, :],
                            lhsT=v_nat[:, mc_, 4 * g + j, :],
                            rhs=pms[j // 2][
                                :, (j % 2) * NHS : (j % 2 + 1) * NHS
                            ],
                            start=(mc_ == 0),
                            stop=(mc_ == MC - 1),
                            tile_position=(0, 32 * j),
                        )

                pv_lag = []
                for mc_ in range(MC):
                    rings = [
                        ps_pool.tile([128, N], F32, tag="ring", name="ring")
                        for _ in range(2)
                    ]
                    for j in range(4):
                        ring = rings[j // 2]
                        col = (j % 2) * NHS
                        nc.tensor.matmul(
                            out=ring[:, col : col + NHS],
                            lhsT=kg[
                                32 * j : 32 * j + KD,
                                mc_ * 128 : (mc_ + 1) * 128,
                            ],
                            rhs=qg[32 * j : 32 * j + KD, ns],
                            start=True,
                            stop=True,
                            tile_position=(32 * j, 0),
                        )
                    pms = []
                    for half in range(2):
                        pT = pt_pool.tile([128, N], BF16, tag="pt", name="pt")
                        nc.scalar.activation(
                            out=pT,
                            in_=rings[half],
                            func=mybir.ActivationFunctionType.Exp,
                            scale=0.25,
                        )
                        pmt = pm_pool.tile([128, N], BF16, tag="pm", name="pm")
                        nc.vector.tensor_mul(
                            pmt,
                            pT,
                            adjT_sb[b][:, mc_, nh * N : (nh + 1) * N],
                        )
                        pms.append(pmt)
                    pv_lag.append((pms, mc_))
                    if len(pv_lag) > 4:
                        args = pv_lag.pop(0)
                        emit_pv(*args)
                    if work_q and mc_ in (1, 3, 5, 7):
                        work_q.popleft()()
                for args in pv_lag:
                    emit_pv(*args)
                nc.vector.tensor_copy(out=hu[g][:, ns], in_=pvt)
                finish_part1(b, g, nh, hu)

        for fn in project_qk(0, 0):
            fn()
        for fn in project_v(0):
            fn()

        def p2(b, g, half, hu, ob, ccs, tail=False):
            return lambda: finish_part2(b, g, half, hu, ob, ccs, tail)

        for b in range(BPC):
            hu = [hu_pool.tile([128, N], F32, tag=f"hu{g}", name=f"hu{g}") for g in range(2)]
            ob = ob_pool.tile([128, MC, E], F32, tag="ob", name="ob")

            if b == 0:
                work_q.extend(project_qk(0, 1))

            run_attention_group(b, 0, hu, ob)

            # g0 finish part2 pieces + next batch's projections, fired
            # piecewise during g1
            work_q.append(p2(b, 0, 0, hu, ob, [0, 1, 2, 3]))
            if b + 1 < BPC:
                work_q.extend(project_qk(b + 1, 0))
            work_q.append(p2(b, 0, 1, hu, ob, [4, 5, 6, 7]))
            if b + 1 < BPC:
                work_q.extend(project_qk(b + 1, 1))
                work_q.extend(project_v(b + 1))

            run_attention_group(b, 1, hu, ob)
            # drain any leftover pieces before the next batch needs them
            while work_q:
                work_q.popleft()()

            if b + 1 < BPC:
                work_q.append(p2(b, 1, 0, hu, ob, [0, 1, 2, 3]))
                work_q.append(p2(b, 1, 1, hu, ob, [4, 5, 6, 7]))
            else:
                while work_q:
                    work_q.popleft()()
                finish_part2(b, 1, 0, hu, ob, tail=True)
                finish_part2(b, 1, 1, hu, ob, tail=True)
        while work_q:
            work_q.popleft()()
    return nc


def _split_multi_waits(nc):
    """walrus codegen in this container allows only one sync-wait per
    instruction; hoist extra waits onto preceding same-engine nops."""
    import copy
    import bass_rust

    tmpl_nc = bass.Bass()
    tmpls = {}
    for en in ["vector", "scalar", "tensor", "gpsimd", "sync"]:
        ins = getattr(tmpl_nc, en).nop().ins
        tmpls[str(ins.engine)] = ins

    uid = [0]
    for fn in nc.m.functions:
        for bb in fn.blocks:
            out = []
            for ins in bb.instructions:
                si = ins.sync_info
                waits = list(si.on_wait) if si is not None else []
                if len(waits) > 1:
                    for w in waits[:-1]:
                        nop = copy.deepcopy(tmpls[str(ins.engine)])
                        uid[0] += 1
                        nop.name = f"I-splitw-{uid[0]}"
                        nop.sync_info = bass_rust.SyncInfo(
                            on_wait=[w], on_update=[]
                        )
                        out.append(nop)
                    ins.sync_info = bass_rust.SyncInfo(
                        on_wait=[waits[-1]], on_update=list(si.on_update)
                    )
                out.append(ins)
            bb.instructions = out
    return nc


_cache = {}


def _get_nc():
    if "nc" not in _cache:
        _cache["nc"] = _split_multi_waits(build_kernel())
    return _cache["nc"]


def kernel(h, adj_c, W_query, W_key, W_val, W_out, trace=False):
    h = np.asarray(h, np.float32)
    adj = np.asarray(adj_c)
    hT = np.ascontiguousarray(
        h.transpose(0, 2, 1).astype(ml_dtypes.bfloat16)
    )  # [B, E, N] bf16
    adjT = adj.transpose(0, 2, 1).astype(ml_dtypes.bfloat16)  # [B, N(m), N(n)]
    # duplicate each n-half so a head-pair's mask is one [128,1024] multiply
    adjT = np.ascontiguousarray(
        np.broadcast_to(
            adjT.reshape(B, N, 2, 1, NHS), (B, N, 2, 2, NHS)
        ).reshape(B, N, 2 * N)
    )
    wq_n = np.asarray(W_query, np.float32)  # [H, E, KD]
    wk_n = np.asarray(W_key, np.float32)
    wq_pad = np.zeros((2, E, 128), np.float32)
    wk_pad = np.zeros((2, E, 128), np.float32)
    for g in range(2):
        for j in range(4):
            wq_pad[g, :, 32 * j : 32 * j + KD] = wq_n[4 * g + j]
            wk_pad[g, :, 32 * j : 32 * j + KD] = wk_n[4 * g + j]
    wv = np.asarray(W_val, np.float32).transpose(1, 0, 2).reshape(E, H * KD)
    wo = np.asarray(W_out, np.float32)
    wo_pad = np.zeros((2, 128, E), np.float32)
    for g in range(2):
        for j in range(4):
            wo_pad[g, 32 * j : 32 * j + KD, :] = wo[4 * g + j]
    bf = ml_dtypes.bfloat16
    wall = np.zeros((128, 7, 128), np.float32)
    wall[:, 0:2, :] = wq_pad.transpose(1, 0, 2)
    wall[:, 2:4, :] = wk_pad.transpose(1, 0, 2)
    wall[:, 4:6, :] = wo_pad.transpose(1, 0, 2)
    wall[:, 6, :] = wv
    wall = np.ascontiguousarray(wall.astype(bf))

    nc = _get_nc()
    in_maps = []
    for c in range(CORES):
        s = slice(c * BPC, (c + 1) * BPC)
        in_maps.append(
            {
                "ht": np.ascontiguousarray(hT[s]),
                "hn": np.ascontiguousarray(h[s]),
                "adjt": np.ascontiguousarray(adjT[s]),
                "wall": wall,
            }
        )
    res = run_bass_kernel_spmd(nc, in_maps, core_ids=list(range(CORES)), trace=trace)
    out = np.concatenate([r["out"] for r in res.results], axis=0)
    if trace:
        return out, res
    return out


# revision 29
# speedup vs baseline: 1.0048x; 1.0048x over previous
"""Multi-head graph attention (GAT-style) Trainium2 Bass kernel.

Full-input contract: kernel(**inputs) takes the complete arrays, shards
batch-wise across 8 NeuronCores (2 batches each), and gathers the output.

Math per batch b, head h (KD=16 head dim):
  Q = h @ Wq_h, K = h @ Wk_h, V = h @ Wv_h            [N, 16]
  compatT[m, n] = (K Q^T)[m, n]                        [N, N] (transposed)
  p = exp(0.25 * compatT) * adjT                       (mask after exp; exact:
      masked entries are exactly 0, matching softmax(-inf) * adj)
  headsT[v, n] = (V'.T @ p)  with V' = [V | 1 | 0*15]  -> row 16 = denominator
  out[n, :] = sum_h (headsT_h / denom_h).T @ Wout_h + h[n, :]

Schedule: the ScalarE exp stream (128 x [128,1024] activations per core,
~1.15us each) is the roofline; everything else hides behind it.
  - compat matmuls 4-way ROW-TILED (contraction 16): heads j=0..3 of group g
    run concurrently in 32-row PE groups (tile_position=(32j,0)); Q/K are
    projected DIRECTLY into gapped layout (head 4g+j at partitions 32j..+15)
    using host-side zero-padded bf16 weights — no shift DMAs.
  - PSUM: ring of 3 x [128,1024] compat tiles (6 banks) + 2 PV banks.
  - loop: b -> g (head group) -> nh (n half) -> mc; PV accumulates over mc
    into pv[g] (col-tiled, head j at partitions 32j, 32-wide with zero pad).
  - finish pipelined per n-half: denominator gather into a [128,32] all-lane
    layout -> reciprocal -> DRAM roundtrip broadcast -> bf16 normalize ->
    bf16 output matmul in the idle PV bank + residual add -> half store.
  - PV matmuls are emitted 4 mc-iterations behind their masks so transient
    DVE work never head-of-line-blocks the in-order PE stream (which would
    stall the exp stream); cross-batch work (next projections, finish
    normalize/output) is queued and fired one small piece per paced mc slot.
  - all input loads issued upfront on the SP HW queue (few big DMAs).

Precision: inputs to all matmuls are bf16 (PSUM accumulation fp32); the
residual add and softmax normalization stay fp32.
"""

import numpy as np
import ml_dtypes
from contextlib import ExitStack

import concourse.bass as bass
import concourse.mybir as mybir
import concourse.tile as tile
from concourse.bass_utils import run_bass_kernel_spmd

B, N, E, H, KD = 16, 1024, 128, 8, 16
CORES = 8
BPC = B // CORES  # batches per core
F32 = mybir.dt.float32
BF16 = mybir.dt.bfloat16
MC = N // 128  # number of 128-row chunks of m
NH = 2         # n halves
NHS = N // NH  # 512
VW = 32        # padded per-head V' width (16 vals + 1 ones + 15 zeros)


def build_kernel():
    nc = bass.Bass()
    hT_d = nc.dram_tensor("ht", [BPC, E, N], BF16, kind="ExternalInput")
    h_d = nc.dram_tensor("hn", [BPC, N, E], F32, kind="ExternalInput")
    adjt_d = nc.dram_tensor("adjt", [BPC, N, 2 * N], BF16, kind="ExternalInput")
    wall_d = nc.dram_tensor("wall", [128, 7, 128], BF16, kind="ExternalInput")
    out_d = nc.dram_tensor("out", [BPC, N, E], F32, kind="ExternalOutput")

    with ExitStack() as ctx:
        tc = ctx.enter_context(tile.TileContext(nc))
        consts = ctx.enter_context(tc.tile_pool(name="consts", bufs=1))
        io_pool = ctx.enter_context(tc.tile_pool(name="io", bufs=2))
        qk_pool = ctx.enter_context(tc.tile_pool(name="qk", bufs=2))
        pt_pool = ctx.enter_context(tc.tile_pool(name="pt", bufs=8))
        pm_pool = ctx.enter_context(tc.tile_pool(name="pm", bufs=8))
        hu_pool = ctx.enter_context(tc.tile_pool(name="hu", bufs=2))
        fin_pool = ctx.enter_context(tc.tile_pool(name="fin", bufs=2))
        ob_pool = ctx.enter_context(tc.tile_pool(name="ob", bufs=2))
        ps_pool = ctx.enter_context(tc.tile_pool(name="ps", bufs=3, space="PSUM"))
        pv_pool = ctx.enter_context(tc.tile_pool(name="pv", bufs=1, space="PSUM"))
        dram = ctx.enter_context(tc.tile_pool(name="dram", bufs=2, space="DRAM"))

        wall_sb = consts.tile([128, 7, 128], BF16, tag="wall")
        wq_sb = [wall_sb[:, g, :] for g in range(2)]
        wk_sb = [wall_sb[:, 2 + g, :] for g in range(2)]
        wo_sb = [wall_sb[:, 4 + g, :] for g in range(2)]
        wv_sb = wall_sb[:, 6, :]

        ones32 = consts.tile([128, 32], BF16, tag="ones32")
        nc.vector.memset(ones32, 1.0)

        # V' tiles with the constant cols preset once (ones col + zero pad);
        # per-batch projections overwrite only cols 0:KD.
        v_fix = []
        for i in range(2):
            vt = consts.tile([128, MC, H, VW], BF16, tag=f"vfix{i}")
            nc.vector.memset(vt[:, :, :, KD : KD + 1], 1.0)
            nc.vector.memset(vt[:, :, :, KD + 1 : VW], 0.0)
            v_fix.append(vt)

        # ---- all input loads upfront (SP HW queue); ordering favors the
        # batch-0 critical path: hT0 + first adj chunk + weights first.
        hT_sb, h_sb, adjT_sb = [], [], []
        for bb in range(BPC):
            ht = io_pool.tile([E, N], BF16, tag="ht", name="ht")
            adj = io_pool.tile([128, MC, 2 * N], BF16, tag="adj", name="adj")
            hn = io_pool.tile([128, MC, E], F32, tag="hn", name="hn")
            hT_sb.append(ht)
            adjT_sb.append(adj)
            h_sb.append(hn)
        nc.sync.dma_start(out=hT_sb[0], in_=hT_d[0, :, :])
        nc.sync.dma_start(out=wall_sb, in_=wall_d[:, :, :])
        nc.sync.dma_start(out=adjT_sb[0][:, 0, :], in_=adjt_d[0, 0:128, :])
        for mc_ in range(1, MC):
            nc.sync.dma_start(
                out=adjT_sb[0][:, mc_, :],
                in_=adjt_d[0, mc_ * 128 : (mc_ + 1) * 128, :],
            )
        nc.sync.dma_start(
            out=h_sb[0], in_=h_d[0].rearrange("(c p) e -> p c e", p=128)
        )
        for bb in range(1, BPC):
            nc.sync.dma_start(out=hT_sb[bb], in_=hT_d[bb, :, :])
            for mc_ in range(MC):
                nc.sync.dma_start(
                    out=adjT_sb[bb][:, mc_, :],
                    in_=adjt_d[bb, mc_ * 128 : (mc_ + 1) * 128, :],
                )
            nc.sync.dma_start(
                out=h_sb[bb], in_=h_d[bb].rearrange("(c p) e -> p c e", p=128)
            )

        qg_all = {}   # (b, g) -> gapped Q tile
        kg_all = {}
        v_all = {}    # b -> v_nat

        def _proj_one(b, g, w_list, store, tagc):
            def p1():
                p_ps = ps_pool.tile([128, N], F32, tag="ring", name="ring")
                for nt in range(2):
                    nc.tensor.matmul(
                        out=p_ps[:, nt * NHS : (nt + 1) * NHS],
                        lhsT=w_list[g],
                        rhs=hT_sb[b][:, nt * NHS : (nt + 1) * NHS],
                        start=True,
                        stop=True,
                    )
                gt = qk_pool.tile([128, N], BF16, tag=f"{tagc}g{g}", name=f"{tagc}g{g}")
                nc.vector.tensor_copy(out=gt, in_=p_ps)
                store[(b, g)] = gt

            return [p1]

        def project_qk(b, g):
            return _proj_one(b, g, wq_sb, qg_all, "q") + _proj_one(
                b, g, wk_sb, kg_all, "k"
            )

        def project_v(b):
            def p1():
                v_ps = ps_pool.tile([128, N], F32, tag="ring", name="ring")
                for mc_ in range(MC):
                    nc.tensor.matmul(
                        out=v_ps[:, mc_ * 128 : (mc_ + 1) * 128],
                        lhsT=hT_sb[b][:, mc_ * 128 : (mc_ + 1) * 128],
                        rhs=wv_sb,
                        start=True,
                        stop=True,
                    )
                v_nat = v_fix[b % 2]
                nc.vector.tensor_copy(
                    out=v_nat[:, :, :, 0:KD],
                    in_=v_ps.rearrange("p (c h k) -> p c h k", h=H, k=KD),
                )
                v_all[b] = v_nat

            return [p1]

        # finish state per (b, g): rbt broadcast tiles per half
        rbts = {}

        def finish_part1(b, g, half, hu, fast_tail=False):
            """denominator gather + reciprocal + broadcast for one n-half.
            dd layout: partition 64*half+16*j+pp, free i (32),
            n = 512*half + 32*pp + i. Normal path broadcasts via a DRAM
            roundtrip into rbt; the tail path (last batch g1) instead gathers
            rec rows to [4,N] and PE-outer-products them into the idle pv0
            bank (shorter latency; ACT queue helps with the DMAs)."""
            key = (b, g)
            if key not in rbts:
                rbts[key] = {
                    "dd": fin_pool.tile([128, 32], F32, tag=f"dd{g}", name=f"dd{g}"),
                }
                if not fast_tail:
                    rbts[key]["rd"] = dram.tile([4, N], F32, tag=f"recd{g}", name=f"recd{g}")
                    rbts[key]["rbt"] = fin_pool.tile([128, N], F32, tag=f"rb{g}", name=f"rb{g}")
            st = rbts[key]
            dd = st["dd"]
            rd, rbt = st.get("rd"), st.get("rbt")
            hs = slice(half * NHS, (half + 1) * NHS)
            post_stream = fast_tail and half == 1
            for j in range(4):
                eng = nc.scalar if (post_stream and j % 2) else nc.sync
                eng.dma_start(
                    out=dd[64 * half + 16 * j : 64 * half + 16 * j + 16, :],
                    in_=hu[g][32 * j + KD : 32 * j + KD + 1, hs].rearrange(
                        "o (p i) -> o p i", p=16
                    ),
                )
            rec = fin_pool.tile([128, 32], F32, tag=f"rec{g}", name=f"rec{g}", bufs=1)
            nc.vector.reciprocal(
                out=rec[64 * half : 64 * half + 64, :],
                in_=dd[64 * half : 64 * half + 64, :],
            )
            if fast_tail:
                recb = fin_pool.tile([128, 32], BF16, tag="recb", name="recb", bufs=1)
                nc.vector.tensor_copy(
                    out=recb[64 * half : 64 * half + 64, :],
                    in_=rec[64 * half : 64 * half + 64, :],
                )
                rec4 = fin_pool.tile([128, NHS], BF16, tag="rec4", name="rec4", bufs=1)
                for j in range(4):
                    eng = nc.scalar if (post_stream and j % 2) else nc.sync
                    eng.dma_start(
                        out=rec4[32 * j : 32 * j + 1, :].rearrange(
                            "o (p i) -> o p i", p=16
                        ),
                        in_=recb[64 * half + 16 * j : 64 * half + 16 * j + 16, :],
                    )
                rbc = pv_pool.tile([128, NHS], F32, tag="pv0", name="rbc")
                for j in range(4):
                    nc.tensor.matmul(
                        out=rbc[32 * j : 32 * j + 32, :],
                        lhsT=ones32[32 * j : 32 * j + 1, :],
                        rhs=rec4[32 * j : 32 * j + 1, :],
                        start=True,
                        stop=True,
                        tile_position=(32 * j, 32 * j),
                    )
                st[f"rbc{half}"] = rbc
                return
            nc.sync.dma_start(
                out=bass.AP(
                    tensor=rd.tensor,
                    offset=rd.offset + half * NHS,
                    ap=[[N, 4], [32, 16], [1, 32]],
                ),
                in_=rec[64 * half : 64 * half + 64, :],
            )
            for j in range(4):
                src = rd[j : j + 1, hs]
                bc = bass.AP(
                    tensor=src.tensor,
                    offset=src.offset,
                    ap=[[0, 32]] + list(src.ap[1:]),
                )
                nc.sync.dma_start(out=rbt[32 * j : 32 * (j + 1), hs], in_=bc)

        def finish_part2(b, g, half, hu, ob, ccs=None, tail=False):
            """normalize (bf16) + output matmul (in the idle pv bank(s)) +
            residual add (g==0) / combine (g==1) into the per-batch ob tile.
            ccs selects a subset of output chunks (normalize runs only when
            the first chunk of the half is included)."""
            hs = slice(half * NHS, (half + 1) * NHS)
            if ccs is None:
                ccs = list(range(4 * half, 4 * half + 4))
            hun = rbts[(b, g)].setdefault(
                "hun",
                fin_pool.tile([128, N], BF16, tag=f"hun{g}", name=f"hun{g}"),
            )
            if 4 * half in ccs:
                rbc = rbts[(b, g)].pop(f"rbc{half}", None)
                if rbc is not None:
                    nc.vector.tensor_mul(hun[:, hs], hu[g][:, hs], rbc)
                else:
                    rbt = rbts[(b, g)]["rbt"]
                    nc.vector.tensor_mul(hun[:, hs], hu[g][:, hs], rbt[:, hs])
            if tail:
                o_pvs = [
                    pv_pool.tile([128, NHS], F32, tag="pv0", name="opv0"),
                    pv_pool.tile([128, NHS], F32, tag="pv1", name="opv1"),
                ]
            else:
                o_pvs = [pv_pool.tile([128, NHS], F32, tag=f"pv{g}", name=f"pv{g}")]
            for i, cc in enumerate(ccs):
                o_pv = o_pvs[i % len(o_pvs)]
                reg = slice((cc % 4) * E, (cc % 4) * E + E)
                nc.tensor.matmul(
                    out=o_pv[:, reg],
                    lhsT=hun[:, cc * 128 : (cc + 1) * 128],
                    rhs=wo_sb[g],
                    start=True,
                    stop=True,
                )
                if g == 0:
                    nc.vector.tensor_add(
                        ob[:, cc, :], o_pv[:, reg], h_sb[b][:, cc, :]
                    )
                else:
                    nc.vector.tensor_add(ob[:, cc, :], o_pv[:, reg], ob[:, cc, :])
            if g == 1 and 4 * half + 3 in ccs:
                nc.sync.dma_start(
                    out=out_d[b, half * NHS : (half + 1) * NHS, :].rearrange(
                        "(c p) e -> p c e", p=128
                    ),
                    in_=ob[:, 4 * half : 4 * half + 4, :],
                )

        # deferred-emission queue: at most one small piece fires per mc
        # iteration, keeping injected PE/DVE work under the per-stage slack
        from collections import deque
        work_q = deque()

        def run_attention_group(b, g, hu, ob):
            qg, kg, v_nat = qg_all[(b, g)], kg_all[(b, g)], v_all[b]
            for nh in range(NH):
                ns = slice(nh * NHS, (nh + 1) * NHS)
                pvt = pv_pool.tile([128, NHS], F32, tag=f"pv{g}", name=f"pv{g}")

                def emit_pv(pms, mc_, pvt=pvt, g=g, v_nat=v_nat):
                    for j in range(4):
                        nc.tensor.matmul(
                            out=pvt[32 * j : 32 * j + VW, :],
                            lhsT=v_nat[:, mc_, 4 * g + j, :],
                            rhs=pms[j // 2][
                                :, (j % 2) * NHS : (j % 2 + 1) * NHS
                            ],
                            start=(mc_ == 0),
                            stop=(mc_ == MC - 1),
                            tile_position=(0, 32 * j),
                        )

                pv_lag = []
                for mc_ in range(MC):
                    rings = [
                        ps_pool.tile([128, N], F32, tag="ring", name="ring")
                        for _ in range(2)
                    ]
                    for j in range(4):
                        ring = rings[j // 2]
                        col = (j % 2) * NHS
                        nc.tensor.matmul(
                            out=ring[:, col : col + NHS],
                            lhsT=kg[
                                32 * j : 32 * j + KD,
                                mc_ * 128 : (mc_ + 1) * 128,
                            ],
                            rhs=qg[32 * j : 32 * j + KD, ns],
                            start=True,
                            stop=True,
                            tile_position=(32 * j, 0),
                        )
                    pms = []
                    for half in range(2):
                        pT = pt_pool.tile([128, N], BF16, tag="pt", name="pt")
                        nc.scalar.activation(
                            out=pT,
                            in_=rings[half],
                            func=mybir.ActivationFunctionType.Exp,
                            scale=0.25,
                        )
                        pmt = pm_pool.tile([128, N], BF16, tag="pm", name="pm")
                        nc.vector.tensor_mul(
                            pmt,
                            pT,
                            adjT_sb[b][:, mc_, nh * N : (nh + 1) * N],
                        )
                        pms.append(pmt)
                    pv_lag.append((pms, mc_))
                    if len(pv_lag) > 4:
                        args = pv_lag.pop(0)
                        emit_pv(*args)
                    if work_q and mc_ in (2, 4, 6):
                        work_q.popleft()()
                for args in pv_lag:
                    emit_pv(*args)
                nc.vector.tensor_copy(out=hu[g][:, ns], in_=pvt)
                finish_part1(b, g, nh, hu)

        for fn in project_qk(0, 0):
            fn()
        for fn in project_v(0):
            fn()

        def p2(b, g, half, hu, ob, ccs, tail=False):
            return lambda: finish_part2(b, g, half, hu, ob, ccs, tail)

        for b in range(BPC):
            hu = [hu_pool.tile([128, N], F32, tag=f"hu{g}", name=f"hu{g}") for g in range(2)]
            ob = ob_pool.tile([128, MC, E], F32, tag="ob", name="ob")

            if b == 0:
                work_q.extend(project_qk(0, 1))

            run_attention_group(b, 0, hu, ob)

            # g0 finish part2 pieces + next batch's projections, fired
            # piecewise during g1
            work_q.append(p2(b, 0, 0, hu, ob, [0, 1, 2, 3]))
            if b + 1 < BPC:
                work_q.extend(project_qk(b + 1, 0))
            work_q.append(p2(b, 0, 1, hu, ob, [4, 5, 6, 7]))
            if b + 1 < BPC:
                work_q.extend(project_qk(b + 1, 1))
                work_q.extend(project_v(b + 1))

            run_attention_group(b, 1, hu, ob)
            # drain any leftover pieces before the next batch needs them
            while work_q:
                work_q.popleft()()

            if b + 1 < BPC:
                work_q.append(p2(b, 1, 0, hu, ob, [0, 1, 2, 3]))
                work_q.append(p2(b, 1, 1, hu, ob, [4, 5, 6, 7]))
            else:
                while work_q:
                    work_q.popleft()()
                finish_part2(b, 1, 0, hu, ob, tail=True)
                finish_part2(b, 1, 1, hu, ob, tail=True)
        while work_q:
            work_q.popleft()()
    return nc


def _split_multi_waits(nc):
    """walrus codegen in this container allows only one sync-wait per
    instruction; hoist extra waits onto preceding same-engine nops."""
    import copy
    import bass_rust

    tmpl_nc = bass.Bass()
    tmpls = {}
    for en in ["vector", "scalar", "tensor", "gpsimd", "sync"]:
        ins = getattr(tmpl_nc, en).nop().ins
        tmpls[str(ins.engine)] = ins

    uid = [0]
    for fn in nc.m.functions:
        for bb in fn.blocks:
            out = []
            for ins in bb.instructions:
                si = ins.sync_info
                waits = list(si.on_wait) if si is not None else []
                if len(waits) > 1:
                    for w in waits[:-1]:
                        nop = copy.deepcopy(tmpls[str(ins.engine)])
                        uid[0] += 1
                        nop.name = f"I-splitw-{uid[0]}"
                        nop.sync_info = bass_rust.SyncInfo(
                            on_wait=[w], on_update=[]
                        )
                        out.append(nop)
                    ins.sync_info = bass_rust.SyncInfo(
                        on_wait=[waits[-1]], on_update=list(si.on_update)
                    )
                out.append(ins)
            bb.instructions = out
    return nc


_cache = {}


def _get_nc():
    if "nc" not in _cache:
        _cache["nc"] = _split_multi_waits(build_kernel())
    return _cache["nc"]


def kernel(h, adj_c, W_query, W_key, W_val, W_out, trace=False):
    h = np.asarray(h, np.float32)
    adj = np.asarray(adj_c)
    hT = np.ascontiguousarray(
        h.transpose(0, 2, 1).astype(ml_dtypes.bfloat16)
    )  # [B, E, N] bf16
    adjT = adj.transpose(0, 2, 1).astype(ml_dtypes.bfloat16)  # [B, N(m), N(n)]
    # duplicate each n-half so a head-pair's mask is one [128,1024] multiply
    adjT = np.ascontiguousarray(
        np.broadcast_to(
            adjT.reshape(B, N, 2, 1, NHS), (B, N, 2, 2, NHS)
        ).reshape(B, N, 2 * N)
    )
    wq_n = np.asarray(W_query, np.float32)  # [H, E, KD]
    wk_n = np.asarray(W_key, np.float32)
    wq_pad = np.zeros((2, E, 128), np.float32)
    wk_pad = np.zeros((2, E, 128), np.float32)
    for g in range(2):
        for j in range(4):
            wq_pad[g, :, 32 * j : 32 * j + KD] = wq_n[4 * g + j]
            wk_pad[g, :, 32 * j : 32 * j + KD] = wk_n[4 * g + j]
    wv = np.asarray(W_val, np.float32).transpose(1, 0, 2).reshape(E, H * KD)
    wo = np.asarray(W_out, np.float32)
    wo_pad = np.zeros((2, 128, E), np.float32)
    for g in range(2):
        for j in range(4):
            wo_pad[g, 32 * j : 32 * j + KD, :] = wo[4 * g + j]
    bf = ml_dtypes.bfloat16
    wall = np.zeros((128, 7, 128), np.float32)
    wall[:, 0:2, :] = wq_pad.transpose(1, 0, 2)
    wall[:, 2:4, :] = wk_pad.transpose(1, 0, 2)
    wall[:, 4:6, :] = wo_pad.transpose(1, 0, 2)
    wall[:, 6, :] = wv
    wall = np.ascontiguousarray(wall.astype(bf))

    nc = _get_nc()
    in_maps = []
    for c in range(CORES):
        s = slice(c * BPC, (c + 1) * BPC)
        in_maps.append(
            {
                "ht": np.ascontiguousarray(hT[s]),
                "hn": np.ascontiguousarray(h[s]),
                "adjt": np.ascontiguousarray(adjT[s]),
                "wall": wall,
            }
        )
    res = run_bass_kernel_spmd(nc, in_maps, core_ids=list(range(CORES)), trace=trace)
    out = np.concatenate([r["out"] for r in res.results], axis=0)
    if trace:
        return out, res
    return out
